# revision 2
# baseline (speedup 1.0000x reference)
"""Sharded cosine-similarity kNN retrieval kernel for Trainium2 (Bass/Tile).

Problem: one query [D] against keys [N, D]; return actions[top_k indices of
cosine similarity].  N=100000, D=2048, A=7, top_k<=8.

Strategy (fp8 TensorEngine scan, DMA-roofline bound):
  - Shard keys row-wise across 8 NeuronCores (12544 rows/core).  Keys are
    downcast to fp8e4m3 on the host and pre-transposed per core into
    [chunk=8, ki=128, ko=2, row] so the device streams them straight into
    the PE array as the DoubleRow moving operand (K=256 per chunk, 2 fp8
    elements per lane per cycle = 2x rate).  This halves HBM traffic vs
    fp16 (25.7 MB/core) and moves all the dot-product math off the
    (previously bottleneck) VectorE onto the otherwise-idle TensorE.
  - Device: for each superblock of 4096 rows, 8x 1MB DMAs (one per
    256-d chunk) feed matmuls q8^T[256,1] @ keysT[256,512] that accumulate
    the 8 chunk partials into psum banks [1,512]; psum -> sbuf copies on
    ScalarE/DVE, one 50KB dots DMA out at the end.
  - Host reduce: take the top CAND=1024 rows by fp8 dot (candidate margin
    is ~20 sigma: fp8 dot err std ~1.7 vs rank8->rank1024 dot gap ~50),
    re-score exactly in fp32 (cosine with norms), pick top_k with
    jax.lax.top_k tie semantics.  The device thus performs the full O(N*D)
    scan; the host only reduces candidates (O(CAND*D)).
"""

import sys

for _p in ("/opt/trn_rl_repo", "/opt/trn_rl_repo/concourse"):
    if _p not in sys.path:
        sys.path.insert(0, _p)

import numpy as np
import ml_dtypes

import concourse.bacc as bacc
from concourse import mybir
from concourse.bass import MemorySpace
from concourse.bass_utils import run_bass_kernel_spmd
from concourse.tile import TileContext

N, D, A = 100000, 2048, 7
EPS = 1e-8
N_CORES = 8
RPC = 12544                  # rows per core; 8*12544 = 100352 >= N
CHUNKS = 8                   # D split into 8 chunks of 256 (DoubleRow K)
NSB = 512                    # rows per matmul / psum bank
SUPER = 4096                 # rows per superblock (8 psum banks)
KBUFS = 16                   # key-tile ring: 2 superblocks of lookahead
CAND = 1024                  # host re-score candidate count
F8 = ml_dtypes.float8_e4m3   # == TRN float8e4 (max 240, inf at S.1111.000)

# superblock list: [(row0, nrows)]
_SUPERS = []
_r = 0
while _r < RPC:
    _SUPERS.append((_r, min(SUPER, RPC - _r)))
    _r += SUPER

_CACHE = {}


def _build_bass(repeats: int = 1, kbufs: int = KBUFS):
    """Build the per-core Bass program.

    repeats>1 wraps the streaming loop in a hardware For loop that re-reads
    the same DRAM shard; used only for wall-clock HW timing (slope over
    repeats cancels host/axon dispatch overhead)."""
    nc = bacc.Bacc(
        "TRN2",
        target_bir_lowering=False,
        debug=False,
        enable_asserts=False,
        num_devices=N_CORES,
    )
    f32 = mybir.dt.float32
    f8 = mybir.dt.float8e4
    keys_d = nc.dram_tensor(
        "keys8", [CHUNKS, 128, 2, RPC], f8, kind="ExternalInput"
    ).ap()
    q_d = nc.dram_tensor("q8", [128, 2, 16], f8, kind="ExternalInput").ap()
    dots_d = nc.dram_tensor("dots", [1, RPC], f32, kind="ExternalOutput").ap()

    with TileContext(nc) as tc:
        with tc.tile_pool(name="kpool", bufs=kbufs) as kpool, \
             tc.tile_pool(name="cpool", bufs=1) as cpool, \
             tc.tile_pool(name="ppool", bufs=8, space=MemorySpace.PSUM) as ppool:
            q_t = cpool.tile([128, 2, 16], f8)
            nc.sync.dma_start(out=q_t, in_=q_d)
            dots_t = cpool.tile([1, RPC], f32)

            def body():
                for r0, nrows in _SUPERS:
                    kts = []
                    for c in range(CHUNKS):
                        kt = kpool.tile([128, 2, SUPER], f8, tag="kt",
                                        name="kt")
                        nc.sync.dma_start(
                            out=kt[:, :, :nrows],
                            in_=keys_d[c, :, :, r0:r0 + nrows],
                        )
                        kts.append(kt)
                    nb = (nrows + NSB - 1) // NSB
                    pts = [ppool.tile([128, NSB], f32, tag="pt", name="pt")
                           for _ in range(nb)]
                    for c in range(CHUNKS):
                        for b in range(nb):
                            ncols = min(NSB, nrows - b * NSB)
                            nc.tensor.matmul(
                                pts[b][0:1, :ncols],
                                q_t[:, :, c:c + 1],
                                kts[c][:, :, b * NSB:b * NSB + ncols],
                                start=(c == 0),
                                stop=(c == CHUNKS - 1),
                                perf_mode=mybir.MatmulPerfMode.DoubleRow,
                            )
                    for b in range(nb):
                        ncols = min(NSB, nrows - b * NSB)
                        nc.any.tensor_copy(
                            dots_t[:, r0 + b * NSB:r0 + b * NSB + ncols],
                            pts[b][0:1, :ncols],
                        )

            if repeats == 1:
                body()
            else:
                with tc.For_i(0, repeats, 1):
                    body()

            nc.sync.dma_start(out=dots_d, in_=dots_t)
    nc.compile()
    return nc


def _get_nc(repeats: int = 1, **kw):
    key = ("nc", repeats, tuple(sorted(kw.items())))
    if key not in _CACHE:
        _CACHE[key] = _build_bass(repeats, **kw)
    return _CACHE[key]


def _pack_keys_shard(keys8_shard: np.ndarray) -> np.ndarray:
    """[rows<=RPC, D] fp8 -> [CHUNKS, 128, 2, RPC]; d = c*256 + ki*2 + ko."""
    rows = keys8_shard.shape[0]
    if rows < RPC:
        pad = np.zeros((RPC, D), dtype=F8)
        pad[:rows] = keys8_shard
        keys8_shard = pad
    # 2-D byte transpose (fast path), then split d -> (c, ki, ko)
    t = np.ascontiguousarray(keys8_shard.view(np.uint8).T)
    return t.view(F8).reshape(CHUNKS, 128, 2, RPC)


def _make_in_maps(keys: np.ndarray, query: np.ndarray):
    q8 = query.astype(F8)
    qarr = np.zeros((128, 2, 16), dtype=F8)
    qarr[:, :, :CHUNKS] = q8.reshape(CHUNKS, 128, 2).transpose(1, 2, 0)
    keys8 = keys.astype(F8)
    in_maps = []
    for i in range(N_CORES):
        lo, hi = i * RPC, min((i + 1) * RPC, N)
        in_maps.append({"keys8": _pack_keys_shard(keys8[lo:hi]), "q8": qarr})
    return in_maps


def _run_device(keys: np.ndarray, query: np.ndarray, trace: bool = False):
    """Run the SPMD kernel; returns (dots[8*RPC] fp8-precision, results)."""
    nc = _get_nc()
    in_maps = _make_in_maps(keys, query)
    res = run_bass_kernel_spmd(
        nc, in_maps, core_ids=list(range(N_CORES)), trace=trace
    )
    dots = np.concatenate(
        [out["dots"][0] for out in res.results])
    return dots, res


def kernel(**inputs) -> np.ndarray:
    query = np.asarray(inputs["query_key"], dtype=np.float32)
    keys = np.asarray(inputs["keys"], dtype=np.float32)
    actions = np.asarray(inputs["actions"])
    top_k = int(inputs["top_k"])
    if top_k <= 0:
        return actions[:0]
    top_k = min(top_k, keys.shape[0])

    dots8, _ = _run_device(keys, query)
    dots8 = dots8[:N]

    # candidate selection by fp8 dot, then exact fp32 cosine re-score
    m = min(max(CAND, 4 * top_k), N)
    cand = np.argpartition(-dots8, m - 1)[:m]
    kc = keys[cand]
    d_ex = kc @ query
    n_ex = np.sqrt((kc * kc).sum(axis=1))
    q_norm = np.float32(np.linalg.norm(query))
    sims_c = d_ex / np.maximum(n_ex * q_norm, np.float32(EPS))

    # top_k among candidates, ties to the lower index (jax.lax.top_k)
    order = np.lexsort((cand, -sims_c))
    idx = cand[order[:top_k]]
    return actions[idx]
